# revision 1
# baseline (speedup 1.0000x reference)
"""GCN VGAE encoder (two GCNConv layers -> (mu, logstd)) on 8 Trainium2
NeuronCores via Bass/Tile — v2.

Math: with deg = 1 + in_degree, dinv = deg^-1/2, and
S(u)[i] = sum_{e: dst e = i} u[src e]:
    v1 = (x @ W1) * dinv
    h  = relu(dinv * (S(v1) + v1) + b1)    v2 = h * dinv
    out = dinv * ((S(v2) + v2) @ [W_mu|W_ls]) + [b_mu|b_ls]
(linear transform commutes with segment_sum; the self-loop term is the
node's own v row, kept in an SBUF slab).

Distribution: nodes + incoming edges sharded over 8 cores by dst range;
v1/v2 AllGathered (bf16, Shared dst) so every core gathers arbitrary src
rows. Aggregation: edges sorted by (bucket of 128 dst rows, src); chunks
of 128 edges; one multi-index indirect DMA gathers GCH*128 rows at a
time; per chunk a one-hot dst matrix (DVE is_equal, bf16) scatters the
gathered rows into the bucket's PSUM via PE matmul.
"""
import numpy as np
import ml_dtypes

import concourse.bass as bass
import concourse.tile as tile
from concourse import mybir
from bass_rust import ScopedClock, SyncInfo

N_NODES = 100000
N_EDGES = 1600000
IN_CH, HID_CH, OUT_CH = 256, 64, 32
N_CORES = 8
NL = N_NODES // N_CORES          # 12500 nodes per core
NB = (NL + 127) // 128           # 98 dst buckets per core
LAST_ROWS = NL - (NB - 1) * 128  # rows in last bucket (84)
GCH = 32                         # chunks per gather instruction

# ---------------------------------------------------------------------------
# Workarounds for the walrus build in this container: it encodes at most ONE
# semaphore wait per instruction and rejects InstIncSwdgeSem.
# ---------------------------------------------------------------------------
_counter = [0]


def _patched_drain_and_barrier(self, tick_clock, wait_clock):
    drain_inst = self.nc.vector.drain()
    wait_clock.add_sem_waits(
        drain_inst.ins, ScopedClock({None: tick_clock.global_clock})
    )
    waits = list(drain_inst.ins.sync_info.on_wait)
    if len(waits) > 1:
        drain_inst.ins.sync_info = SyncInfo(on_wait=[waits[0]], on_update=[])
        for w in waits[1:]:
            extra = self.nc.vector.drain()
            extra.ins.sync_info = SyncInfo(on_wait=[w], on_update=[])
    self.nc.all_engine_barrier()
    assert self.sems is not None
    popped = self.nc._tile_sem_poison_stack.pop()
    assert popped is self._sem_poison
    self.nc.clear_and_free_semaphores(list(self.sems.allocated().values()))
    self.nc.all_engine_barrier()


tile.TileContext._drain_and_barrier = _patched_drain_and_barrier


def _steal_sem_clear(nc, first, last):
    cur = nc.cur_bb.bb
    inst = nc.gpsimd.sem_clear(range(first, last + 1))
    il = cur.instructions
    assert il and il[-1] is inst.ins
    cur.instructions = il[:-1]
    return inst.ins


def _fix_incswdge(nc):
    for bb in nc.main_func.blocks:
        il = bb.instructions
        if not any(type(i).__name__ == "InstIncSwdgeSem" for i in il):
            continue
        new_list = []
        for ins in il:
            if type(ins).__name__ != "InstIncSwdgeSem":
                new_list.append(ins)
                continue
            base = ins._sem_id_base
            values = list(ins._sem_values)
            names = list(ins._sem_names)
            si = ins.sync_info
            waits = list(si.on_wait) if si is not None else []
            for w in waits:
                _counter[0] += 1
                nop = mybir.InstNoOp(name=f"SWF-{_counter[0]}", ins=[], outs=[])
                nop.engine = ins.engine
                nop.sync_info = SyncInfo(on_wait=[w], on_update=[])
                new_list.append(nop)
            if ins._mode == "sub":
                nz = [k for k, v in enumerate(values) if v]
                if nz:
                    new_list.append(_steal_sem_clear(nc, base + min(nz), base + max(nz)))
            else:
                for k, v in enumerate(values):
                    for _ in range(v):
                        _counter[0] += 1
                        nop = mybir.InstNoOp(name=f"SWF-{_counter[0]}", ins=[], outs=[])
                        nop.engine = ins.engine
                        nop.sync_info = SyncInfo(
                            on_wait=[],
                            on_update=[
                                mybir.SyncUpdate(
                                    sync_type="semaphore", id=base + k,
                                    ant_name=names[k], update_mode="sem-inc",
                                    update_value=v * 0 + 1,
                                )
                            ],
                        )
                        new_list.append(nop)
        bb.instructions = new_list


def _split_multiwaits(nc):
    for bb in nc.main_func.blocks:
        il = bb.instructions
        if not any(
            i.sync_info is not None and len(i.sync_info.on_wait) > 1 for i in il
        ):
            continue
        new_list = []
        for ins in il:
            si = ins.sync_info
            waits = list(si.on_wait) if si is not None else []
            if len(waits) > 1:
                ups = list(si.on_update)
                for w in waits[:-1]:
                    _counter[0] += 1
                    nop = mybir.InstNoOp(name=f"WSP-{_counter[0]}", ins=[], outs=[])
                    nop.engine = ins.engine
                    nop.sync_info = SyncInfo(on_wait=[w], on_update=[])
                    new_list.append(nop)
                ins.sync_info = SyncInfo(on_wait=[waits[-1]], on_update=ups)
            new_list.append(ins)
        bb.instructions = new_list


# ---------------------------------------------------------------------------
# Device program
# ---------------------------------------------------------------------------
def _build_program(cpbs, reps=1, variant=""):
    """variant: "" normal; "noag" tiny dummy collectives (timing attribution);
    "nogather_sw" static-AP SWDGE loads instead of indirect gathers;
    "nogather_hw" static-AP HWDGE (sync engine) loads instead."""
    cpbs = list(cpbs)
    C = sum(cpbs)
    cstart = np.zeros(NB + 1, np.int64)
    np.cumsum(cpbs, out=cstart[1:])
    bucket_of = np.repeat(np.arange(NB), cpbs)

    f32, bf16, i32 = mybir.dt.float32, mybir.dt.bfloat16, mybir.dt.int32
    nq = 4 if variant == "q4" else 1
    nc = bass.Bass("TRN2", target_bir_lowering=False, debug=False,
                   num_devices=N_CORES, num_swdge_queues=nq)

    xT = nc.dram_tensor("xT", [NB, 128, IN_CH], bf16, kind="ExternalInput")
    w1 = nc.dram_tensor("w1", [IN_CH, HID_CH], bf16, kind="ExternalInput")
    wmuls = nc.dram_tensor("wmuls", [HID_CH, 2 * OUT_CH], f32, kind="ExternalInput")
    b1b = nc.dram_tensor("b1b", [128, HID_CH], f32, kind="ExternalInput")
    bmlb = nc.dram_tensor("bmlb", [128, 2 * OUT_CH], f32, kind="ExternalInput")
    dinvw = nc.dram_tensor("dinvw", [128, NB], f32, kind="ExternalInput")
    iota_in = nc.dram_tensor("iota_in", [128, 128], i32, kind="ExternalInput")
    ident_in = nc.dram_tensor("ident_in", [128, 128], f32, kind="ExternalInput")
    srcw = nc.dram_tensor("srcw", [128, C], i32, kind="ExternalInput")
    dstw = nc.dram_tensor("dstw", [128, C], i32, kind="ExternalInput")
    out = nc.dram_tensor("out", [NL, 2 * OUT_CH], f32, kind="ExternalOutput")

    u1b = nc.dram_tensor("u1b", [NL, HID_CH], bf16)
    u1g = nc.dram_tensor("u1g", [N_NODES, HID_CH], bf16, addr_space="Shared")
    u2b = nc.dram_tensor("u2b", [NL, HID_CH], bf16)
    u2g = nc.dram_tensor("u2g", [N_NODES, HID_CH], bf16, addr_space="Shared")

    def allgather(src, dst):
        if variant == "noag":
            # tiny dummy collective: keeps phase->pass sync structure while
            # making collective time negligible
            nc.gpsimd.collective_compute(
                "AllGather", mybir.AluOpType.bypass,
                replica_groups=[list(range(N_CORES))],
                ins=[src[0:8, :].opt()], outs=[dst[0:64, :].opt()],
            )
        else:
            nc.gpsimd.collective_compute(
                "AllGather", mybir.AluOpType.bypass,
                replica_groups=[list(range(N_CORES))],
                ins=[src[:].opt()], outs=[dst[:].opt()],
            )

    gathers = [(c0, min(GCH, C - c0)) for c0 in range(0, C, GCH)]

    with tile.TileContext(nc) as tc:
        with (
            tc.tile_pool(name="const", bufs=1) as cp,
            tc.tile_pool(name="slab", bufs=1) as sp,
        ):
            w1_sb = [cp.tile([128, HID_CH], bf16, name=f"w1sb{k}")
                     for k in range(IN_CH // 128)]
            for k in range(IN_CH // 128):
                nc.sync.dma_start(out=w1_sb[k][:],
                                  in_=w1[k * 128:(k + 1) * 128, :])
            wml_sb = cp.tile([HID_CH, 2 * OUT_CH], f32)
            nc.sync.dma_start(out=wml_sb[:], in_=wmuls[:])
            b1_sb = cp.tile([128, HID_CH], f32)
            nc.sync.dma_start(out=b1_sb[:], in_=b1b[:])
            bml_sb = cp.tile([128, 2 * OUT_CH], f32)
            nc.sync.dma_start(out=bml_sb[:], in_=bmlb[:])
            dinv_sb = cp.tile([128, NB], f32)
            nc.sync.dma_start(out=dinv_sb[:], in_=dinvw[:])
            iota = cp.tile([128, 128], i32)
            nc.sync.dma_start(out=iota[:], in_=iota_in[:])
            ident = cp.tile([128, 128], f32)
            nc.sync.dma_start(out=ident[:], in_=ident_in[:])
            src_sb = sp.tile([128, C], i32)
            nc.sync.dma_start(out=src_sb[:], in_=srcw[:])
            dst_sb = sp.tile([128, C], i32)
            nc.sync.dma_start(out=dst_sb[:], in_=dstw[:])
            u1_slab = sp.tile([128, NB * HID_CH], bf16)
            u2_slab = sp.tile([128, NB * HID_CH], bf16)

            iota_b = iota[:, :].unsqueeze(1)

            for _ in range(reps):
                # phase 1: v1 = (x @ W1) * dinv
                with (
                    tc.tile_pool(name="xload", bufs=4) as xp,
                    tc.tile_pool(name="vout", bufs=4) as vp,
                    tc.tile_pool(name="psv", bufs=2, space="PSUM") as ppv,
                ):
                    for m in range(NB):
                        rows = 128 if m < NB - 1 else LAST_ROWS
                        xt2 = xp.tile([128, IN_CH], bf16, tag="x")
                        (nc.sync if m % 2 == 0 else nc.scalar).dma_start(
                            out=xt2[:], in_=xT[m])
                        v_ps = ppv.tile([128, HID_CH], f32, tag="v")
                        for k in range(IN_CH // 128):
                            nc.tensor.matmul(
                                out=v_ps[:],
                                lhsT=xt2[:, k * 128:(k + 1) * 128],
                                rhs=w1_sb[k][:],
                                start=(k == 0), stop=(k == IN_CH // 128 - 1),
                            )
                        v_t = u1_slab[:, m * HID_CH:(m + 1) * HID_CH]
                        nc.vector.tensor_scalar_mul(
                            v_t, v_ps[:], dinv_sb[:, m:m + 1])
                        (nc.scalar if m % 2 == 0 else nc.sync).dma_start(
                            out=u1b[m * 128:m * 128 + rows, :], in_=v_t[:rows, :])

                allgather(u1b, u1g)

                def aggregation_pass(table, second):
                    pools = [
                        tc.tile_pool(name="ut", bufs=16),
                        tc.tile_pool(name="mt", bufs=3),
                        tc.tile_pool(name="ep", bufs=4),
                        tc.tile_pool(name="psg", bufs=4, space="PSUM"),
                    ]
                    if second:
                        pools.append(tc.tile_pool(name="pst", bufs=2, space="PSUM"))
                        pools.append(tc.tile_pool(name="pso", bufs=2, space="PSUM"))
                    ctxs = [p.__enter__() for p in pools]
                    up, mp, ep, ppg = ctxs[:4]
                    if second:
                        ppt, ppo = ctxs[4], ctxs[5]
                    g_ps = None
                    u_fix = None
                    if variant == "noload":
                        u_fix = up.tile([128, HID_CH], bf16, tag="ufix")
                        nc.vector.memset(u_fix[:], 0.5)
                    for c0, n in gathers:
                        m_t = mp.tile([128, GCH * 128], bf16, tag="m")
                        m_view = m_t[:, :n * 128].rearrange("p (c i) -> p c i", c=n)
                        nc.vector.tensor_tensor(
                            out=m_view,
                            in0=dst_sb[:, c0:c0 + n].to_broadcast([128, n, 128]),
                            in1=iota_b.broadcast_to([128, n, 128]),
                            op=mybir.AluOpType.is_equal,
                        )
                        for j in range(n):
                            c = c0 + j
                            b = int(bucket_of[c])
                            first = (c == cstart[b])
                            last = (c == cstart[b + 1] - 1)
                            if variant == "noload":
                                u_t = u_fix
                            else:
                                u_t = up.tile([128, HID_CH], bf16, tag="u")
                            if variant == "noload":
                                pass
                            elif variant in ("nogather_sw", "nogather_hw"):
                                r0 = (c % (N_NODES // 128 - 1)) * 128
                                eng = (nc.gpsimd if variant == "nogather_sw"
                                       else nc.sync)
                                eng.dma_start(out=u_t[:],
                                              in_=table[r0:r0 + 128, :])
                            else:
                                nc.gpsimd.indirect_dma_start(
                                    out=u_t[:], out_offset=None, in_=table[:],
                                    in_offset=bass.IndirectOffsetOnAxis(
                                        ap=src_sb[:, c:c + 1], axis=0),
                                )
                            if first:
                                g_ps = ppg.tile([128, HID_CH], f32, tag="g")
                            nc.tensor.matmul(
                                out=g_ps[:],
                                lhsT=m_t[:, j * 128:(j + 1) * 128],
                                rhs=u_t[:],
                                start=first, stop=last,
                            )
                            if not last:
                                continue
                            rows = 128 if b < NB - 1 else LAST_ROWS
                            if not second:
                                t = ep.tile([128, HID_CH], f32, tag="t")
                                nc.vector.tensor_add(
                                    out=t[:], in0=g_ps[:],
                                    in1=u1_slab[:, b * HID_CH:(b + 1) * HID_CH])
                                nc.vector.tensor_scalar_mul(
                                    t[:], t[:], dinv_sb[:, b:b + 1])
                                nc.vector.tensor_add(out=t[:], in0=t[:], in1=b1_sb[:])
                                nc.scalar.activation(
                                    t[:], t[:], mybir.ActivationFunctionType.Relu)
                                u2_t = u2_slab[:, b * HID_CH:(b + 1) * HID_CH]
                                nc.vector.tensor_scalar_mul(
                                    u2_t, t[:], dinv_sb[:, b:b + 1])
                                (nc.scalar if b % 2 == 0 else nc.sync).dma_start(
                                    out=u2b[b * 128:b * 128 + rows, :],
                                    in_=u2_t[:rows, :])
                            else:
                                t = ep.tile([128, HID_CH], f32, tag="t")
                                nc.vector.tensor_add(
                                    out=t[:], in0=g_ps[:],
                                    in1=u2_slab[:, b * HID_CH:(b + 1) * HID_CH])
                                nc.vector.tensor_scalar_mul(
                                    t[:], t[:], dinv_sb[:, b:b + 1])
                                tT_ps = ppt.tile([HID_CH, 128], f32, tag="tT")
                                nc.tensor.transpose(
                                    out=tT_ps[:], in_=t[:], identity=ident[:])
                                tT_sb = ep.tile([HID_CH, 128], f32, tag="tTs")
                                nc.scalar.copy(out=tT_sb[:], in_=tT_ps[:])
                                o_ps = ppo.tile([128, 2 * OUT_CH], f32, tag="o")
                                nc.tensor.matmul(
                                    out=o_ps[:], lhsT=tT_sb[:], rhs=wml_sb[:],
                                    start=True, stop=True,
                                )
                                o_sb = ep.tile([128, 2 * OUT_CH], f32, tag="os")
                                nc.vector.tensor_add(
                                    out=o_sb[:], in0=o_ps[:], in1=bml_sb[:])
                                (nc.scalar if b % 2 == 0 else nc.sync).dma_start(
                                    out=out[b * 128:b * 128 + rows, :],
                                    in_=o_sb[:rows, :])
                    for p in reversed(pools):
                        p.__exit__(None, None, None)

                aggregation_pass(u1g, second=False)

                allgather(u2b, u2g)

                aggregation_pass(u2g, second=True)

    if variant == "sp1":
        for bb_ in nc.main_func.blocks:
            for ins in bb_.instructions:
                if (type(ins).__name__ == "InstDMACopy"
                        and getattr(ins, "queue", None) == "qPoolDynamic"
                        and any(getattr(a, "dynamic_ap_info", None) is not None
                                for a in ins.ins
                                if hasattr(a, "dynamic_ap_info"))):
                    ins.single_packet = True

    if variant == "q4":
        # round-robin the indirect gathers across the 4 SWDGE queue rings
        k = 0
        for bb_ in nc.main_func.blocks:
            for ins in bb_.instructions:
                if (type(ins).__name__ == "InstDMACopy"
                        and getattr(ins, "queue", None) == "qPoolDynamic"
                        and any(getattr(a, "dynamic_ap_info", None) is not None
                                for a in ins.ins
                                if hasattr(a, "dynamic_ap_info"))):
                    ins.queue = f"qPoolDynamic{k % 4 or ''}"
                    k += 1

    _fix_incswdge(nc)
    _split_multiwaits(nc)
    return nc


# ---------------------------------------------------------------------------
# Host-side sharding + launch
# ---------------------------------------------------------------------------
_cache = {}


def _prep(x, edge_index, W1, b1, W_mu, b_mu, W_ls, b_ls):
    x = np.asarray(x, np.float32)
    ei = np.asarray(edge_index)
    src = ei[0].astype(np.int64)
    dst = ei[1].astype(np.int64)

    deg = np.bincount(dst, minlength=N_NODES).astype(np.float32) + 1.0
    dinv = (1.0 / np.sqrt(deg)).astype(np.float32)

    s = src
    d = dst

    core = d // NL
    drel = d - core * NL
    bucket = drel >> 7
    d128 = (drel & 127).astype(np.int32)

    gb = core * NB + bucket
    counts = np.bincount(gb, minlength=N_CORES * NB).reshape(N_CORES, NB)
    cpbs = np.maximum(
        (counts.max(axis=0) + 127) // 128, 1).astype(np.int64)  # [NB]
    cstart = np.zeros(NB + 1, np.int64)
    np.cumsum(cpbs, out=cstart[1:])
    C = int(cstart[-1])

    order = np.lexsort((s, bucket, core))
    gb_s = gb[order]
    gstarts = np.zeros(N_CORES * NB + 1, np.int64)
    np.cumsum(counts.ravel(), out=gstarts[1:])
    rank = np.arange(len(order), dtype=np.int64) - gstarts[gb_s]
    cc = gb_s // NB
    bb = gb_s % NB
    slot = cstart[bb] * 128 + rank

    src_w = np.zeros((N_CORES, C * 128), np.int32)
    dst_w = np.full((N_CORES, C * 128), -1, np.int32)
    src_w[cc, slot] = s[order].astype(np.int32)
    dst_w[cc, slot] = d128[order]
    src_w = np.ascontiguousarray(
        src_w.reshape(N_CORES, C, 128).transpose(0, 2, 1))
    dst_w = np.ascontiguousarray(
        dst_w.reshape(N_CORES, C, 128).transpose(0, 2, 1))

    # x tiles: xT[m, p, kb*128+j] = x_core[m*128+j, kb*128+p]
    xs = x.reshape(N_CORES, NL, IN_CH)
    xpad = np.zeros((N_CORES, NB * 128, IN_CH), np.float32)
    xpad[:, :NL] = xs
    xT = np.ascontiguousarray(
        xpad.reshape(N_CORES, NB, 128, IN_CH // 128, 128)
        .transpose(0, 1, 4, 3, 2)
        .reshape(N_CORES, NB, 128, IN_CH)).astype(ml_dtypes.bfloat16)

    dinvw = np.ones((N_CORES, 128, NB), np.float32)
    dv = dinv.reshape(N_CORES, NL)
    for b in range(NB):
        rows = 128 if b < NB - 1 else LAST_ROWS
        dinvw[:, :rows, b] = dv[:, b * 128:b * 128 + rows]

    wmuls = np.concatenate([np.asarray(W_mu, np.float32),
                            np.asarray(W_ls, np.float32)], axis=1)
    bml = np.concatenate([np.asarray(b_mu, np.float32),
                          np.asarray(b_ls, np.float32)])[None, :]
    in_map_common = {
        "w1": np.asarray(W1, np.float32).astype(ml_dtypes.bfloat16),
        "wmuls": wmuls,
        "b1b": np.broadcast_to(np.asarray(b1, np.float32)[None, :],
                               (128, HID_CH)).copy(),
        "bmlb": np.broadcast_to(bml, (128, 2 * OUT_CH)).copy(),
        "iota_in": np.broadcast_to(np.arange(128, dtype=np.int32)[None, :],
                                   (128, 128)).copy(),
        "ident_in": np.eye(128, dtype=np.float32),
    }
    in_maps = []
    for c in range(N_CORES):
        m = dict(in_map_common)
        m["xT"] = xT[c]
        m["srcw"] = src_w[c]
        m["dstw"] = dst_w[c]
        m["dinvw"] = dinvw[c]
        in_maps.append(m)
    return tuple(cpbs.tolist()), in_maps


def kernel(x, edge_index, W1, b1, W_mu, b_mu, W_ls, b_ls):
    from concourse.bass_utils import run_bass_kernel_spmd

    key, in_maps = _prep(x, edge_index, W1, b1, W_mu, b_mu, W_ls, b_ls)
    if key not in _cache:
        _cache[key] = _build_program(key)
    nc = _cache[key]
    res = run_bass_kernel_spmd(nc, in_maps, list(range(N_CORES)))
    full = np.concatenate([res.results[c]["out"] for c in range(N_CORES)], axis=0)
    return full[:, :OUT_CH].copy(), full[:, OUT_CH:].copy()

